# revision 5
# baseline (speedup 1.0000x reference)
"""Trainium2 Bass kernel for nn_Better_Transformer (block-diagonal MLP + supact + residual).

Math (per reference):
    x_norm = x * gain + norm_bias
    y = blockdiag_matmul(x_norm, W) + bias          # 32 blocks of 128x128
    mult = gamma + sigmoid(beta * y) * (1 - gamma)
    out = mult * y + x

Strategy:
  - Data-parallel over batch: 16384 rows -> 8 cores x 2048 rows.
  - Host folds gain/norm_bias into W and bias:  W' = gain*W,
    bias' = bias + norm_bias * colsum(W').
  - On device, compute in transposed space (features on partitions) so all
    per-column constants become per-partition scalars:
      PE-transpose x blocks (fp32, exact) -> ACT evac to float32r ->
      f32r matmul (full PE rate) + K=1 bias matmul accumulate ->
      ACT Sigmoid(scale=beta_p) -> GPSIMD m = s*(1-g)+g -> DVE o = m*y ->
      PE-transpose back -> DVE residual add (fused PSUM evac) -> DMA out.
"""
import sys

for _p in ("/opt/trn_rl_repo", "/root/.axon_site/_ro/trn_rl_repo"):
    if _p not in sys.path:
        sys.path.insert(0, _p)

import numpy as np
from contextlib import ExitStack

import concourse.bacc as bacc
import concourse.tile as tile
from concourse import mybir
from concourse import bass_utils

# problem shapes (hardcoded)
BATCH = 16384
IN_SIZE = 4096
N_PART = 32
INT_DIM = 128
N_CORES = 8
ROWS = BATCH // N_CORES          # 2048 rows per core
CHUNK = 512                      # rows per pipeline chunk
N_CHUNK = ROWS // CHUNK          # 4
TPC = CHUNK // 128               # 4 b-tiles (128 rows) per chunk

F32 = mybir.dt.float32
F32R = mybir.dt.float32r
AF = mybir.ActivationFunctionType
ALU = mybir.AluOpType

# engine that computes m = s*(1-gamma)+gamma: "gpsimd" or "vector"
M_ENGINE = "gpsimd"


def build_program(repeat=1):
    nc = bacc.Bacc("TRN2", target_bir_lowering=False, debug=False)

    x_d = nc.dram_tensor("x", (ROWS, IN_SIZE), F32, kind="ExternalInput").ap()
    wt_d = nc.dram_tensor("wt", (128, IN_SIZE), F32, kind="ExternalInput").ap()
    br_d = nc.dram_tensor("bias_row", (1, IN_SIZE), F32, kind="ExternalInput").ap()
    cons_d = nc.dram_tensor("cons", (128, 3 * N_PART), F32, kind="ExternalInput").ap()
    id_d = nc.dram_tensor("ident", (128, 128), F32, kind="ExternalInput").ap()
    out_d = nc.dram_tensor("out", (ROWS, IN_SIZE), F32, kind="ExternalOutput").ap()

    with ExitStack() as ctx:
        tc = ctx.enter_context(tile.TileContext(nc))

        # ---- constants: load fp32 scratch, round to f32r, release scratch
        cpool = ctx.enter_context(tc.tile_pool(name="consts", bufs=1))
        with tc.tile_pool(name="scratch", bufs=1) as scratch:
            w_f = scratch.tile([128, IN_SIZE], F32)
            nc.sync.dma_start(w_f[:], wt_d[:])
            br_f = scratch.tile([1, IN_SIZE], F32)
            nc.sync.dma_start(br_f[:], br_d[:])
            ones_f = scratch.tile([1, CHUNK], F32)
            nc.vector.memset(ones_f[:], 1.0)

            wr = cpool.tile([128, IN_SIZE], F32R)
            nc.vector.tensor_copy(wr[:], w_f[:])
            brr = cpool.tile([1, IN_SIZE], F32R)
            nc.vector.tensor_copy(brr[:], br_f[:])
            onesr = cpool.tile([1, CHUNK], F32R)
            nc.vector.tensor_copy(onesr[:], ones_f[:])

        cons_sb = cpool.tile([128, 3 * N_PART], F32)
        nc.sync.dma_start(cons_sb[:], cons_d[:])
        id_sb = cpool.tile([128, 128], F32)
        nc.sync.dma_start(id_sb[:], id_d[:])

        beta_c = lambda p: cons_sb[:, p:p + 1]
        gamma_c = lambda p: cons_sb[:, N_PART + p:N_PART + p + 1]
        omg_c = lambda p: cons_sb[:, 2 * N_PART + p:2 * N_PART + p + 1]

        # ---- pools
        xpool = ctx.enter_context(tc.tile_pool(name="xin", bufs=5))
        opool = ctx.enter_context(tc.tile_pool(name="oout", bufs=4))
        xtp = ctx.enter_context(tc.tile_pool(name="xt", bufs=3))
        smp = ctx.enter_context(tc.tile_pool(name="sm", bufs=2))
        ogp = ctx.enter_context(tc.tile_pool(name="og", bufs=6))
        psp = ctx.enter_context(tc.tile_pool(name="ps", bufs=2, space="PSUM"))

        m_eng = getattr(nc, M_ENGINE)

        rep_ctx = tc.For_i(0, repeat, 1) if repeat > 1 else None
        if rep_ctx is not None:
            rep_ctx.__enter__()

        for c in range(N_CHUNK):
            x_tiles = []
            for i in range(TPC):
                xt = xpool.tile([128, IN_SIZE], F32, tag="x", name=f"x_{c}_{i}")
                r0 = (c * TPC + i) * 128
                nc.sync.dma_start(xt[:], x_d[r0:r0 + 128, :])
                x_tiles.append(xt)
            out_tiles = [
                opool.tile([128, IN_SIZE], F32, tag="out", name=f"out_{c}_{i}")
                for i in range(TPC)
            ]

            o_group = []
            for p in range(N_PART):
                ps_xt = psp.tile([128, CHUNK], F32, tag="ps_xt", name=f"psxt_{c}_{p}")
                for i in range(TPC):
                    nc.tensor.transpose(
                        ps_xt[:, i * 128:(i + 1) * 128],
                        x_tiles[i][:, p * 128:(p + 1) * 128],
                        id_sb[:],
                    )
                xt_sb = xtp.tile([128, CHUNK], F32R, tag="xts", name=f"xts_{c}_{p}")
                nc.scalar.copy(xt_sb[:], ps_xt[:])

                ps_y = psp.tile([128, CHUNK], F32, tag="ps_y", name=f"psy_{c}_{p}")
                nc.tensor.matmul(ps_y[:], wr[:, p * 128:(p + 1) * 128], xt_sb[:],
                                 start=True, stop=False)
                nc.tensor.matmul(ps_y[:], brr[:, p * 128:(p + 1) * 128], onesr[:],
                                 start=False, stop=True)

                s_sb = smp.tile([128, CHUNK], F32, tag="s", name=f"s_{c}_{p}")
                nc.scalar.activation(s_sb[:], ps_y[:], AF.Sigmoid, scale=beta_c(p))

                m_sb = smp.tile([128, CHUNK], F32, tag="m", name=f"m_{c}_{p}")
                m_eng.tensor_scalar(m_sb[:], s_sb[:], omg_c(p), gamma_c(p),
                                    ALU.mult, ALU.add)

                o_sb = ogp.tile([128, CHUNK], F32, tag="o", name=f"o_{c}_{p}")
                nc.vector.tensor_tensor(o_sb[:], m_sb[:], ps_y[:], ALU.mult)
                o_group.append(o_sb)

                if p % 4 == 3:
                    q = p // 4
                    for i in range(TPC):
                        ps_og = psp.tile([128, 512], F32, tag="ps_og",
                                         name=f"psog_{c}_{q}_{i}")
                        for j in range(4):
                            nc.tensor.transpose(
                                ps_og[:, j * 128:(j + 1) * 128],
                                o_group[j][:, i * 128:(i + 1) * 128],
                                id_sb[:],
                            )
                        nc.vector.tensor_tensor(
                            out_tiles[i][:, q * 512:(q + 1) * 512],
                            ps_og[:],
                            x_tiles[i][:, q * 512:(q + 1) * 512],
                            ALU.add,
                        )
                    o_group = []

            for i in range(TPC):
                r0 = (c * TPC + i) * 128
                nc.sync.dma_start(out_d[r0:r0 + 128, :], out_tiles[i][:])

        if rep_ctx is not None:
            rep_ctx.__exit__(None, None, None)

    nc.finalize()
    return nc


def kernel(x, weights, bias, gain, norm_bias, gamma, beta, **_ignored):
    x = np.ascontiguousarray(np.asarray(x, dtype=np.float32))
    weights = np.asarray(weights, dtype=np.float32)
    bias = np.asarray(bias, dtype=np.float32)
    gain = np.asarray(gain, dtype=np.float32)
    norm_bias = np.asarray(norm_bias, dtype=np.float32)
    gamma = np.asarray(gamma, dtype=np.float32)
    beta = np.asarray(beta, dtype=np.float32)

    # host-side constant folding
    W = weights * float(gain.reshape(-1)[0])                      # [P, D, D]
    colsum = W.sum(axis=1)                                        # [P, D]
    bias1 = bias + float(norm_bias.reshape(-1)[0]) * colsum.reshape(-1)
    wt = np.ascontiguousarray(W.transpose(1, 0, 2).reshape(128, IN_SIZE))
    bias_row = np.ascontiguousarray(bias1.reshape(1, IN_SIZE))
    beta_p = beta.reshape(N_PART, 128).T                          # [128, P]
    gamma_p = gamma.reshape(N_PART, 128).T
    omg_p = (1.0 - gamma).reshape(N_PART, 128).T
    cons = np.ascontiguousarray(
        np.concatenate([beta_p, gamma_p, omg_p], axis=1).astype(np.float32))
    ident = np.eye(128, dtype=np.float32)

    nc = build_program()

    in_maps = []
    for core in range(N_CORES):
        shard = x[core * ROWS:(core + 1) * ROWS]
        in_maps.append({
            "x": shard,
            "wt": wt,
            "bias_row": bias_row,
            "cons": cons,
            "ident": ident,
        })

    res = bass_utils.run_bass_kernel_spmd(nc, in_maps, core_ids=list(range(N_CORES)))
    out = np.concatenate([res.results[i]["out"] for i in range(N_CORES)], axis=0)
    return out


if __name__ == "__main__":
    xs = np.random.randn(BATCH, IN_SIZE).astype(np.float32)
    ws = np.random.randn(N_PART, INT_DIM, INT_DIM).astype(np.float32) / 11.3
    out = kernel(
        x=xs, weights=ws,
        bias=np.zeros(IN_SIZE, np.float32),
        gain=np.ones(1, np.float32),
        norm_bias=np.zeros(1, np.float32),
        gamma=np.ones(IN_SIZE, np.float32),
        beta=np.zeros(IN_SIZE, np.float32),
    )
    print(out.shape, out.dtype)


# revision 8
# speedup vs baseline: 1.1837x; 1.1837x over previous
"""Trainium2 Bass kernel for nn_Better_Transformer (block-diagonal MLP + supact + residual).

Math (per reference):
    x_norm = x * gain + norm_bias
    y = blockdiag_matmul(x_norm, W) + bias          # 32 blocks of 128x128
    mult = gamma + sigmoid(beta * y) * (1 - gamma)
    out = mult * y + x

Strategy:
  - Data-parallel over batch: 16384 rows -> 8 cores x 2048 rows.
  - Host folds gain/norm_bias into W and bias:  W' = gain*W,
    bias' = bias + norm_bias * colsum(W').
  - On device, compute in transposed space (features on partitions) so all
    per-column constants become per-partition scalars:
      PE-transpose x blocks (fp32, exact) -> ACT evac to float32r ->
      f32r matmul (full PE rate) + K=1 bias matmul accumulate ->
      ACT Sigmoid(scale=beta_p) -> GPSIMD m = s*(1-g)+g -> DVE o = m*y ->
      PE-transpose back -> DVE residual add (fused PSUM evac) -> DMA out.
"""
import sys

for _p in ("/opt/trn_rl_repo", "/root/.axon_site/_ro/trn_rl_repo"):
    if _p not in sys.path:
        sys.path.insert(0, _p)

import numpy as np
from contextlib import ExitStack

import concourse.bacc as bacc
import concourse.tile as tile
from concourse import mybir
from concourse import bass_utils

# problem shapes (hardcoded)
BATCH = 16384
IN_SIZE = 4096
N_PART = 32
INT_DIM = 128
N_CORES = 8
ROWS = BATCH // N_CORES          # 2048 rows per core
CHUNK = 512                      # rows per pipeline chunk
N_CHUNK = ROWS // CHUNK          # 4
TPC = CHUNK // 128               # 4 b-tiles (128 rows) per chunk

F32 = mybir.dt.float32
F32R = mybir.dt.float32r
AF = mybir.ActivationFunctionType
ALU = mybir.AluOpType

# engine that computes m = s*(1-gamma)+gamma: "gpsimd" or "vector"
M_ENGINE = "gpsimd"


def build_program(repeat=1):
    nc = bacc.Bacc("TRN2", target_bir_lowering=False, debug=False)

    x_d = nc.dram_tensor("x", (ROWS, IN_SIZE), F32, kind="ExternalInput").ap()
    wt_d = nc.dram_tensor("wt", (128, IN_SIZE), F32, kind="ExternalInput").ap()
    br_d = nc.dram_tensor("bias_row", (1, IN_SIZE), F32, kind="ExternalInput").ap()
    cons_d = nc.dram_tensor("cons", (128, 3 * N_PART), F32, kind="ExternalInput").ap()
    id_d = nc.dram_tensor("ident", (128, 128), F32, kind="ExternalInput").ap()
    out_d = nc.dram_tensor("out", (ROWS, IN_SIZE), F32, kind="ExternalOutput").ap()

    with ExitStack() as ctx:
        tc = ctx.enter_context(tile.TileContext(nc))

        # ---- constants: load fp32 scratch, round to f32r, release scratch
        cpool = ctx.enter_context(tc.tile_pool(name="consts", bufs=1))
        with tc.tile_pool(name="scratch", bufs=1) as scratch:
            w_f = scratch.tile([128, IN_SIZE], F32)
            nc.sync.dma_start(w_f[:], wt_d[:])
            br_f = scratch.tile([1, IN_SIZE], F32)
            nc.sync.dma_start(br_f[:], br_d[:])
            ones_f = scratch.tile([1, CHUNK], F32)
            nc.vector.memset(ones_f[:], 1.0)

            wr = cpool.tile([128, IN_SIZE], F32R)
            nc.vector.tensor_copy(wr[:], w_f[:])
            brr = cpool.tile([1, IN_SIZE], F32R)
            nc.vector.tensor_copy(brr[:], br_f[:])
            onesr = cpool.tile([1, CHUNK], F32R)
            nc.vector.tensor_copy(onesr[:], ones_f[:])

        cons_sb = cpool.tile([128, 3 * N_PART], F32)
        nc.sync.dma_start(cons_sb[:], cons_d[:])
        id_sb = cpool.tile([128, 128], F32)
        nc.sync.dma_start(id_sb[:], id_d[:])

        beta_c = lambda p: cons_sb[:, p:p + 1]
        gamma_c = lambda p: cons_sb[:, N_PART + p:N_PART + p + 1]
        omg_c = lambda p: cons_sb[:, 2 * N_PART + p:2 * N_PART + p + 1]

        # ---- pools
        xpool = ctx.enter_context(tc.tile_pool(name="xin", bufs=8))
        opool = ctx.enter_context(tc.tile_pool(name="oout", bufs=8))
        xtp = ctx.enter_context(tc.tile_pool(name="xt", bufs=3))
        smp = ctx.enter_context(tc.tile_pool(name="sm", bufs=2))
        ogp = ctx.enter_context(tc.tile_pool(name="og", bufs=6))
        psp = ctx.enter_context(tc.tile_pool(name="ps", bufs=2, space="PSUM"))

        m_eng = getattr(nc, M_ENGINE)

        rep_ctx = tc.For_i(0, repeat, 1) if repeat > 1 else None
        if rep_ctx is not None:
            rep_ctx.__enter__()

        for c in range(N_CHUNK):
            x_tiles = []
            for i in range(TPC):
                xt = xpool.tile([128, IN_SIZE], F32, tag="x", name=f"x_{c}_{i}")
                r0 = (c * TPC + i) * 128
                nc.sync.dma_start(xt[:], x_d[r0:r0 + 128, :])
                x_tiles.append(xt)

            o_group = []
            for p in range(N_PART):
                ps_xt = psp.tile([128, CHUNK], F32, tag="ps_xt", name=f"psxt_{c}_{p}")
                for i in range(TPC):
                    nc.tensor.transpose(
                        ps_xt[:, i * 128:(i + 1) * 128],
                        x_tiles[i][:, p * 128:(p + 1) * 128],
                        id_sb[:],
                    )
                xt_sb = xtp.tile([128, CHUNK], F32R, tag="xts", name=f"xts_{c}_{p}")
                nc.scalar.copy(xt_sb[:], ps_xt[:])

                ps_y = psp.tile([128, CHUNK], F32, tag="ps_y", name=f"psy_{c}_{p}")
                nc.tensor.matmul(ps_y[:], wr[:, p * 128:(p + 1) * 128], xt_sb[:],
                                 start=True, stop=False)
                nc.tensor.matmul(ps_y[:], brr[:, p * 128:(p + 1) * 128], onesr[:],
                                 start=False, stop=True)

                s_sb = smp.tile([128, CHUNK], F32, tag="s", name=f"s_{c}_{p}")
                nc.scalar.activation(s_sb[:], ps_y[:], AF.Sigmoid, scale=beta_c(p))

                m_sb = smp.tile([128, CHUNK], F32, tag="m", name=f"m_{c}_{p}")
                m_eng.tensor_scalar(m_sb[:], s_sb[:], omg_c(p), gamma_c(p),
                                    ALU.mult, ALU.add)

                o_sb = ogp.tile([128, CHUNK], F32, tag="o", name=f"o_{c}_{p}")
                nc.vector.tensor_tensor(o_sb[:], m_sb[:], ps_y[:], ALU.mult)
                o_group.append(o_sb)

                if p % 4 == 3:
                    q = p // 4
                    for i in range(TPC):
                        ps_og = psp.tile([128, 512], F32, tag="ps_og",
                                         name=f"psog_{c}_{q}_{i}")
                        for j in range(4):
                            nc.tensor.transpose(
                                ps_og[:, j * 128:(j + 1) * 128],
                                o_group[j][:, i * 128:(i + 1) * 128],
                                id_sb[:],
                            )
                        res_sb = opool.tile([128, 512], F32, tag="res",
                                            name=f"res_{c}_{q}_{i}")
                        nc.vector.tensor_tensor(
                            res_sb[:],
                            ps_og[:],
                            x_tiles[i][:, q * 512:(q + 1) * 512],
                            ALU.add,
                        )
                        r0 = (c * TPC + i) * 128
                        nc.sync.dma_start(
                            out_d[r0:r0 + 128, q * 512:(q + 1) * 512], res_sb[:])
                    o_group = []

        if rep_ctx is not None:
            rep_ctx.__exit__(None, None, None)

    nc.finalize()
    return nc


def kernel(x, weights, bias, gain, norm_bias, gamma, beta, **_ignored):
    x = np.ascontiguousarray(np.asarray(x, dtype=np.float32))
    weights = np.asarray(weights, dtype=np.float32)
    bias = np.asarray(bias, dtype=np.float32)
    gain = np.asarray(gain, dtype=np.float32)
    norm_bias = np.asarray(norm_bias, dtype=np.float32)
    gamma = np.asarray(gamma, dtype=np.float32)
    beta = np.asarray(beta, dtype=np.float32)

    # host-side constant folding
    W = weights * float(gain.reshape(-1)[0])                      # [P, D, D]
    colsum = W.sum(axis=1)                                        # [P, D]
    bias1 = bias + float(norm_bias.reshape(-1)[0]) * colsum.reshape(-1)
    wt = np.ascontiguousarray(W.transpose(1, 0, 2).reshape(128, IN_SIZE))
    bias_row = np.ascontiguousarray(bias1.reshape(1, IN_SIZE))
    beta_p = beta.reshape(N_PART, 128).T                          # [128, P]
    gamma_p = gamma.reshape(N_PART, 128).T
    omg_p = (1.0 - gamma).reshape(N_PART, 128).T
    cons = np.ascontiguousarray(
        np.concatenate([beta_p, gamma_p, omg_p], axis=1).astype(np.float32))
    ident = np.eye(128, dtype=np.float32)

    nc = build_program()

    in_maps = []
    for core in range(N_CORES):
        shard = x[core * ROWS:(core + 1) * ROWS]
        in_maps.append({
            "x": shard,
            "wt": wt,
            "bias_row": bias_row,
            "cons": cons,
            "ident": ident,
        })

    res = bass_utils.run_bass_kernel_spmd(nc, in_maps, core_ids=list(range(N_CORES)))
    out = np.concatenate([res.results[i]["out"] for i in range(N_CORES)], axis=0)
    return out


if __name__ == "__main__":
    xs = np.random.randn(BATCH, IN_SIZE).astype(np.float32)
    ws = np.random.randn(N_PART, INT_DIM, INT_DIM).astype(np.float32) / 11.3
    out = kernel(
        x=xs, weights=ws,
        bias=np.zeros(IN_SIZE, np.float32),
        gain=np.ones(1, np.float32),
        norm_bias=np.zeros(1, np.float32),
        gamma=np.ones(IN_SIZE, np.float32),
        beta=np.zeros(IN_SIZE, np.float32),
    )
    print(out.shape, out.dtype)


# revision 20
# speedup vs baseline: 1.8725x; 1.5819x over previous
"""Trainium2 Bass kernel for nn_Better_Transformer (block-diagonal MLP + supact + residual).

Math (per reference):
    x_norm = x * gain + norm_bias
    y = blockdiag_matmul(x_norm, W) + bias          # 32 blocks of 128x128
    mult = gamma + sigmoid(beta * y) * (1 - gamma)
    out = mult * y + x

Strategy:
  - Data-parallel over batch: 16384 rows -> 8 cores x 2048 rows.
  - Host folds gain/norm_bias into W and bias:  W' = gain*W,
    bias' = bias + norm_bias * colsum(W').
  - On device, compute in transposed space (features on partitions) so all
    per-column constants become per-partition scalars:
      PE-transpose x blocks (fp32, exact) -> ACT evac to float32r ->
      f32r matmul (full PE rate) + K=1 bias matmul accumulate ->
      ACT Sigmoid(scale=beta_p) -> GPSIMD m = s*(1-g)+g -> DVE o = m*y ->
      PE-transpose back -> DVE residual add (fused PSUM evac) -> DMA out.
"""
import sys

for _p in ("/opt/trn_rl_repo", "/root/.axon_site/_ro/trn_rl_repo"):
    if _p not in sys.path:
        sys.path.insert(0, _p)

import numpy as np
from contextlib import ExitStack

import concourse.bacc as bacc
import concourse.tile as tile
from concourse import mybir
from concourse import bass_utils

# problem shapes (hardcoded)
BATCH = 16384
IN_SIZE = 4096
N_PART = 32
INT_DIM = 128
N_CORES = 8
ROWS = BATCH // N_CORES          # 2048 rows per core
CHUNK = 512                      # rows per pipeline chunk
N_CHUNK = ROWS // CHUNK          # 4
TPC = CHUNK // 128               # 4 b-tiles (128 rows) per chunk

F32 = mybir.dt.float32
F32R = mybir.dt.float32r
AF = mybir.ActivationFunctionType
ALU = mybir.AluOpType

# engine that computes m = s*(1-gamma)+gamma: "gpsimd" or "vector"
M_ENGINE = "gpsimd"


def build_program(repeat=1):
    nc = bacc.Bacc("TRN2", target_bir_lowering=False, debug=False)

    x_d = nc.dram_tensor("x", (ROWS, IN_SIZE), F32, kind="ExternalInput").ap()
    wt_d = nc.dram_tensor("wt", (128, IN_SIZE), F32, kind="ExternalInput").ap()
    cons_d = nc.dram_tensor("cons", (128, 5 * N_PART), F32, kind="ExternalInput").ap()
    id_d = nc.dram_tensor("ident", (128, 128), F32, kind="ExternalInput").ap()
    out_d = nc.dram_tensor("out", (ROWS, IN_SIZE), F32, kind="ExternalOutput").ap()

    with ExitStack() as ctx:
        tc = ctx.enter_context(tile.TileContext(nc))

        # ---- constants: load fp32 scratch, round to f32r, release scratch
        cpool = ctx.enter_context(tc.tile_pool(name="consts", bufs=1))
        with tc.tile_pool(name="scratch", bufs=1) as scratch:
            w_f = scratch.tile([128, IN_SIZE], F32)
            nc.sync.dma_start(w_f[:], wt_d[:])
            wr = cpool.tile([128, IN_SIZE], F32R)
            nc.vector.tensor_copy(wr[:], w_f[:])

        cons_sb = cpool.tile([128, 5 * N_PART], F32)
        nc.sync.dma_start(cons_sb[:], cons_d[:])
        id_sb = cpool.tile([128, 128], F32)
        nc.sync.dma_start(id_sb[:], id_d[:])

        # cons columns: [beta, gamma, 1-gamma, beta*bias1, bias1] per p
        beta_c = lambda p: cons_sb[:, p:p + 1]
        gamma_c = lambda p: cons_sb[:, N_PART + p:N_PART + p + 1]
        omg_c = lambda p: cons_sb[:, 2 * N_PART + p:2 * N_PART + p + 1]
        bb_c = lambda p: cons_sb[:, 3 * N_PART + p:3 * N_PART + p + 1]
        bias_c = lambda p: cons_sb[:, 4 * N_PART + p:4 * N_PART + p + 1]

        # ---- pools
        xpool = ctx.enter_context(tc.tile_pool(name="xin", bufs=8))
        opool = ctx.enter_context(tc.tile_pool(name="oout", bufs=8))
        xtp = ctx.enter_context(tc.tile_pool(name="xt", bufs=4))
        smp = ctx.enter_context(tc.tile_pool(name="sm", bufs=4))
        ogp = ctx.enter_context(tc.tile_pool(name="og", bufs=6))
        psp = ctx.enter_context(tc.tile_pool(name="ps", bufs=3, space="PSUM"))

        m_eng = getattr(nc, M_ENGINE)

        rep_ctx = tc.For_i(0, repeat, 1) if repeat > 1 else None
        if rep_ctx is not None:
            rep_ctx.__enter__()

        def load_chunk(c):
            tiles = []
            for i in range(TPC):
                xt = xpool.tile([128, IN_SIZE], F32, tag="x", name=f"x_{c}_{i}")
                r0 = (c * TPC + i) * 128
                for qt in range(4):
                    c0 = qt * 1024
                    nc.sync.dma_start(xt[:, c0:c0 + 1024],
                                      x_d[r0:r0 + 128, c0:c0 + 1024])
                tiles.append(xt)
            return tiles

        x_tiles_next = load_chunk(0)
        for c in range(N_CHUNK):
            x_tiles = x_tiles_next
            o_group = []
            for p in range(N_PART):
                if p == 8 and c + 1 < N_CHUNK:
                    x_tiles_next = load_chunk(c + 1)
                ps_xt = psp.tile([128, CHUNK], F32, tag="ps_xt", name=f"psxt_{c}_{p}")
                for i in range(TPC):
                    nc.tensor.transpose(
                        ps_xt[:, i * 128:(i + 1) * 128],
                        x_tiles[i][:, p * 128:(p + 1) * 128],
                        id_sb[:],
                    )
                xt_sb = xtp.tile([128, CHUNK], F32R, tag="xts", name=f"xts_{c}_{p}")
                nc.scalar.copy(xt_sb[:], ps_xt[:])

                ps_y = psp.tile([128, CHUNK], F32, tag="ps_y", name=f"psy_{c}_{p}")
                nc.tensor.matmul(ps_y[:], wr[:, p * 128:(p + 1) * 128], xt_sb[:],
                                 start=True, stop=True)

                # s = sigmoid(beta*(yraw + bias1)) = sigmoid(beta*yraw + beta*bias1)
                s_sb = smp.tile([128, CHUNK], F32, tag="s", name=f"s_{c}_{p}")
                nc.scalar.activation(s_sb[:], ps_y[:], AF.Sigmoid,
                                     scale=beta_c(p), bias=bb_c(p))

                m_sb = smp.tile([128, CHUNK], F32, tag="m", name=f"m_{c}_{p}")
                m_eng.tensor_scalar(m_sb[:], s_sb[:], omg_c(p), gamma_c(p),
                                    ALU.mult, ALU.add)

                # o = (yraw + bias1) * m   (fused bias add + gate)
                o_sb = ogp.tile([128, CHUNK], F32, tag="o", name=f"o_{c}_{p}")
                nc.vector.scalar_tensor_tensor(o_sb[:], ps_y[:], bias_c(p), m_sb[:],
                                               ALU.add, ALU.mult)

                o_group.append(o_sb)
                if p % 4 == 3:
                    q = p // 4
                    for i in range(TPC):
                        ps_og = psp.tile([128, 512], F32, tag="ps_og", bufs=2,
                                         name=f"psog_{c}_{q}_{i}")
                        for j in range(4):
                            nc.tensor.transpose(
                                ps_og[:, j * 128:(j + 1) * 128],
                                o_group[j][:, i * 128:(i + 1) * 128],
                                id_sb[:],
                            )
                        res_sb = opool.tile([128, 512], F32, tag="res",
                                            name=f"res_{c}_{q}_{i}")
                        nc.vector.tensor_tensor(
                            res_sb[:],
                            ps_og[:],
                            x_tiles[i][:, q * 512:(q + 1) * 512],
                            ALU.add,
                        )
                        r0 = (c * TPC + i) * 128
                        nc.sync.dma_start(
                            out_d[r0:r0 + 128, q * 512:(q + 1) * 512], res_sb[:])
                    o_group = []

        if rep_ctx is not None:
            rep_ctx.__exit__(None, None, None)

    nc.finalize()
    return nc


def fold_constants(weights, bias, gain, norm_bias, gamma, beta):
    W = weights * float(np.reshape(gain, -1)[0])                  # [P, D, D]
    colsum = W.sum(axis=1)                                        # [P, D]
    bias1 = (bias + float(np.reshape(norm_bias, -1)[0]) * colsum.reshape(-1)).astype(np.float32)
    wt = np.ascontiguousarray(W.transpose(1, 0, 2).reshape(128, IN_SIZE)).astype(np.float32)
    beta_p = beta.reshape(N_PART, 128).T                          # [128, P]
    gamma_p = gamma.reshape(N_PART, 128).T
    omg_p = (1.0 - gamma).reshape(N_PART, 128).T
    bb_p = (beta * bias1).reshape(N_PART, 128).T
    bias_p = bias1.reshape(N_PART, 128).T
    cons = np.ascontiguousarray(
        np.concatenate([beta_p, gamma_p, omg_p, bb_p, bias_p], axis=1).astype(np.float32))
    ident = np.eye(128, dtype=np.float32)
    return wt, cons, ident


def kernel(x, weights, bias, gain, norm_bias, gamma, beta, **_ignored):
    x = np.ascontiguousarray(np.asarray(x, dtype=np.float32))
    weights = np.asarray(weights, dtype=np.float32)
    bias = np.asarray(bias, dtype=np.float32)
    gain = np.asarray(gain, dtype=np.float32)
    norm_bias = np.asarray(norm_bias, dtype=np.float32)
    gamma = np.asarray(gamma, dtype=np.float32)
    beta = np.asarray(beta, dtype=np.float32)

    # host-side constant folding
    wt, cons, ident = fold_constants(weights, bias, gain, norm_bias, gamma, beta)

    nc = build_program()

    in_maps = []
    for core in range(N_CORES):
        shard = x[core * ROWS:(core + 1) * ROWS]
        in_maps.append({
            "x": shard,
            "wt": wt,
            "cons": cons,
            "ident": ident,
        })

    res = bass_utils.run_bass_kernel_spmd(nc, in_maps, core_ids=list(range(N_CORES)))
    out = np.concatenate([res.results[i]["out"] for i in range(N_CORES)], axis=0)
    return out


if __name__ == "__main__":
    xs = np.random.randn(BATCH, IN_SIZE).astype(np.float32)
    ws = np.random.randn(N_PART, INT_DIM, INT_DIM).astype(np.float32) / 11.3
    out = kernel(
        x=xs, weights=ws,
        bias=np.zeros(IN_SIZE, np.float32),
        gain=np.ones(1, np.float32),
        norm_bias=np.zeros(1, np.float32),
        gamma=np.ones(IN_SIZE, np.float32),
        beta=np.zeros(IN_SIZE, np.float32),
    )
    print(out.shape, out.dtype)


# revision 25
# speedup vs baseline: 2.0712x; 1.1061x over previous
"""Trainium2 Bass kernel for nn_Better_Transformer (block-diagonal MLP + supact + residual).

Math (per reference):
    x_norm = x * gain + norm_bias
    y = blockdiag_matmul(x_norm, W) + bias          # 32 blocks of 128x128
    mult = gamma + sigmoid(beta * y) * (1 - gamma)
    out = mult * y + x

Strategy:
  - Data-parallel over batch: 16384 rows -> 8 cores x 2048 rows.
  - Host folds gain/norm_bias into W and bias:  W' = gain*W,
    bias' = bias + norm_bias * colsum(W').
  - On device, compute in transposed space (features on partitions) so all
    per-column constants become per-partition scalars:
      PE-transpose x blocks (fp32, exact) -> ACT evac to float32r ->
      f32r matmul (full PE rate; yraw in PSUM) ->
      ACT Sigmoid(scale=beta_p, bias=beta_p*bias'_p) ->
      GPSIMD m = s*(1-g)+g ->
      DVE o = (yraw + bias'_p) * m  (scalar_tensor_tensor, fused bias add) ->
      PE-transpose back -> DVE residual add (fused PSUM evac) ->
      streamed [128,512] output DMAs.
  - Measured ~189 us/core: ~100% of the 358 GB/s per-core HBM roofline
    (64.5 MiB of unavoidable fp32 I/O per core).
"""
import sys

for _p in ("/opt/trn_rl_repo", "/root/.axon_site/_ro/trn_rl_repo"):
    if _p not in sys.path:
        sys.path.insert(0, _p)

import numpy as np
from contextlib import ExitStack

import concourse.bacc as bacc
import concourse.tile as tile
from concourse import mybir
from concourse import bass_utils

# problem shapes (hardcoded)
BATCH = 16384
IN_SIZE = 4096
N_PART = 32
INT_DIM = 128
N_CORES = 8
ROWS = BATCH // N_CORES          # 2048 rows per core
CHUNK = 512                      # rows per pipeline chunk
N_CHUNK = ROWS // CHUNK          # 4
TPC = CHUNK // 128               # 4 b-tiles (128 rows) per chunk

F32 = mybir.dt.float32
F32R = mybir.dt.float32r
AF = mybir.ActivationFunctionType
ALU = mybir.AluOpType

# engine that computes m = s*(1-gamma)+gamma: "gpsimd" or "vector"
M_ENGINE = "gpsimd"
PREFETCH_P = 28  # p index at which next chunk's loads are emitted


def build_program(repeat=1):
    nc = bacc.Bacc("TRN2", target_bir_lowering=False, debug=False)

    x_d = nc.dram_tensor("x", (ROWS, IN_SIZE), F32, kind="ExternalInput").ap()
    wt_d = nc.dram_tensor("wt", (128, IN_SIZE), F32, kind="ExternalInput").ap()
    cons_d = nc.dram_tensor("cons", (128, 5 * N_PART), F32, kind="ExternalInput").ap()
    id_d = nc.dram_tensor("ident", (128, 128), F32, kind="ExternalInput").ap()
    out_d = nc.dram_tensor("out", (ROWS, IN_SIZE), F32, kind="ExternalOutput").ap()

    with ExitStack() as ctx:
        tc = ctx.enter_context(tile.TileContext(nc))

        # ---- constants: load fp32 scratch, round to f32r, release scratch
        cpool = ctx.enter_context(tc.tile_pool(name="consts", bufs=1))
        with tc.tile_pool(name="scratch", bufs=1) as scratch:
            w_f = scratch.tile([128, IN_SIZE], F32)
            nc.sync.dma_start(w_f[:], wt_d[:])
            wr = cpool.tile([128, IN_SIZE], F32R)
            nc.vector.tensor_copy(wr[:], w_f[:])

        cons_sb = cpool.tile([128, 5 * N_PART], F32)
        nc.sync.dma_start(cons_sb[:], cons_d[:])
        id_sb = cpool.tile([128, 128], F32)
        nc.sync.dma_start(id_sb[:], id_d[:])

        # cons columns: [beta, gamma, 1-gamma, beta*bias1, bias1] per p
        beta_c = lambda p: cons_sb[:, p:p + 1]
        gamma_c = lambda p: cons_sb[:, N_PART + p:N_PART + p + 1]
        omg_c = lambda p: cons_sb[:, 2 * N_PART + p:2 * N_PART + p + 1]
        bb_c = lambda p: cons_sb[:, 3 * N_PART + p:3 * N_PART + p + 1]
        bias_c = lambda p: cons_sb[:, 4 * N_PART + p:4 * N_PART + p + 1]

        # ---- pools
        xpool = ctx.enter_context(tc.tile_pool(name="xin", bufs=8))
        opool = ctx.enter_context(tc.tile_pool(name="oout", bufs=8))
        xtp = ctx.enter_context(tc.tile_pool(name="xt", bufs=4))
        smp = ctx.enter_context(tc.tile_pool(name="sm", bufs=4))
        ogp = ctx.enter_context(tc.tile_pool(name="og", bufs=6))
        psp = ctx.enter_context(tc.tile_pool(name="ps", bufs=3, space="PSUM"))

        m_eng = getattr(nc, M_ENGINE)

        rep_ctx = tc.For_i(0, repeat, 1) if repeat > 1 else None
        if rep_ctx is not None:
            rep_ctx.__enter__()

        def load_chunk(c):
            tiles = [
                xpool.tile([128, IN_SIZE], F32, tag="x", name=f"x_{c}_{i}")
                for i in range(TPC)
            ]
            for qt in range(4):
                c0 = qt * 1024
                for i in range(TPC):
                    r0 = (c * TPC + i) * 128
                    nc.sync.dma_start(tiles[i][:, c0:c0 + 1024],
                                      x_d[r0:r0 + 128, c0:c0 + 1024])
            return tiles

        x_tiles_next = load_chunk(0)
        for c in range(N_CHUNK):
            x_tiles = x_tiles_next
            o_group = []
            for p in range(N_PART):
                if p == PREFETCH_P and c + 1 < N_CHUNK:
                    x_tiles_next = load_chunk(c + 1)
                ps_xt = psp.tile([128, CHUNK], F32, tag="ps_xt", name=f"psxt_{c}_{p}")
                for i in range(TPC):
                    nc.tensor.transpose(
                        ps_xt[:, i * 128:(i + 1) * 128],
                        x_tiles[i][:, p * 128:(p + 1) * 128],
                        id_sb[:],
                    )
                xt_sb = xtp.tile([128, CHUNK], F32R, tag="xts", name=f"xts_{c}_{p}")
                nc.scalar.copy(xt_sb[:], ps_xt[:])

                ps_y = psp.tile([128, CHUNK], F32, tag="ps_y", name=f"psy_{c}_{p}")
                nc.tensor.matmul(ps_y[:], wr[:, p * 128:(p + 1) * 128], xt_sb[:],
                                 start=True, stop=True)

                # s = sigmoid(beta*(yraw + bias1)) = sigmoid(beta*yraw + beta*bias1)
                s_sb = smp.tile([128, CHUNK], F32, tag="s", name=f"s_{c}_{p}")
                nc.scalar.activation(s_sb[:], ps_y[:], AF.Sigmoid,
                                     scale=beta_c(p), bias=bb_c(p))

                m_sb = smp.tile([128, CHUNK], F32, tag="m", name=f"m_{c}_{p}")
                m_eng.tensor_scalar(m_sb[:], s_sb[:], omg_c(p), gamma_c(p),
                                    ALU.mult, ALU.add)

                # o = (yraw + bias1) * m   (fused bias add + gate)
                o_sb = ogp.tile([128, CHUNK], F32, tag="o", name=f"o_{c}_{p}")
                nc.vector.scalar_tensor_tensor(o_sb[:], ps_y[:], bias_c(p), m_sb[:],
                                               ALU.add, ALU.mult)

                o_group.append(o_sb)
                if p % 4 == 3:
                    q = p // 4
                    for i in range(TPC):
                        ps_og = psp.tile([128, 512], F32, tag="ps_og", bufs=2,
                                         name=f"psog_{c}_{q}_{i}")
                        for j in range(4):
                            nc.tensor.transpose(
                                ps_og[:, j * 128:(j + 1) * 128],
                                o_group[j][:, i * 128:(i + 1) * 128],
                                id_sb[:],
                            )
                        res_sb = opool.tile([128, 512], F32, tag="res",
                                            name=f"res_{c}_{q}_{i}")
                        nc.vector.tensor_tensor(
                            res_sb[:],
                            ps_og[:],
                            x_tiles[i][:, q * 512:(q + 1) * 512],
                            ALU.add,
                        )
                        r0 = (c * TPC + i) * 128
                        nc.sync.dma_start(
                            out_d[r0:r0 + 128, q * 512:(q + 1) * 512], res_sb[:])
                    o_group = []

        if rep_ctx is not None:
            rep_ctx.__exit__(None, None, None)

    nc.finalize()
    return nc


def fold_constants(weights, bias, gain, norm_bias, gamma, beta):
    W = weights * float(np.reshape(gain, -1)[0])                  # [P, D, D]
    colsum = W.sum(axis=1)                                        # [P, D]
    bias1 = (bias + float(np.reshape(norm_bias, -1)[0]) * colsum.reshape(-1)).astype(np.float32)
    wt = np.ascontiguousarray(W.transpose(1, 0, 2).reshape(128, IN_SIZE)).astype(np.float32)
    beta_p = beta.reshape(N_PART, 128).T                          # [128, P]
    gamma_p = gamma.reshape(N_PART, 128).T
    omg_p = (1.0 - gamma).reshape(N_PART, 128).T
    bb_p = (beta * bias1).reshape(N_PART, 128).T
    bias_p = bias1.reshape(N_PART, 128).T
    cons = np.ascontiguousarray(
        np.concatenate([beta_p, gamma_p, omg_p, bb_p, bias_p], axis=1).astype(np.float32))
    ident = np.eye(128, dtype=np.float32)
    return wt, cons, ident


def kernel(x, weights, bias, gain, norm_bias, gamma, beta, **_ignored):
    x = np.ascontiguousarray(np.asarray(x, dtype=np.float32))
    weights = np.asarray(weights, dtype=np.float32)
    bias = np.asarray(bias, dtype=np.float32)
    gain = np.asarray(gain, dtype=np.float32)
    norm_bias = np.asarray(norm_bias, dtype=np.float32)
    gamma = np.asarray(gamma, dtype=np.float32)
    beta = np.asarray(beta, dtype=np.float32)

    # host-side constant folding
    wt, cons, ident = fold_constants(weights, bias, gain, norm_bias, gamma, beta)

    nc = build_program()

    in_maps = []
    for core in range(N_CORES):
        shard = x[core * ROWS:(core + 1) * ROWS]
        in_maps.append({
            "x": shard,
            "wt": wt,
            "cons": cons,
            "ident": ident,
        })

    res = bass_utils.run_bass_kernel_spmd(nc, in_maps, core_ids=list(range(N_CORES)))
    out = np.concatenate([res.results[i]["out"] for i in range(N_CORES)], axis=0)
    return out


if __name__ == "__main__":
    xs = np.random.randn(BATCH, IN_SIZE).astype(np.float32)
    ws = np.random.randn(N_PART, INT_DIM, INT_DIM).astype(np.float32) / 11.3
    out = kernel(
        x=xs, weights=ws,
        bias=np.zeros(IN_SIZE, np.float32),
        gain=np.ones(1, np.float32),
        norm_bias=np.zeros(1, np.float32),
        gamma=np.ones(IN_SIZE, np.float32),
        beta=np.zeros(IN_SIZE, np.float32),
    )
    print(out.shape, out.dtype)


# revision 29
# speedup vs baseline: 2.6081x; 1.2592x over previous
"""Trainium2 Bass kernel for nn_Better_Transformer (block-diagonal MLP + supact + residual).

Math (per reference):
    x_norm = x * gain + norm_bias
    y = blockdiag_matmul(x_norm, W) + bias          # 32 blocks of 128x128
    mult = gamma + sigmoid(beta * y) * (1 - gamma)
    out = mult * y + x

Strategy:
  - Data-parallel over batch: 16384 rows -> 8 cores x 2048 rows.
  - Host folds gain/norm_bias into W and bias:  W' = gain*W,
    bias' = bias + norm_bias * colsum(W').
  - On device, compute in transposed space (features on partitions) so all
    per-column constants become per-partition scalars:
      PE-transpose x blocks (fp32, exact) -> ACT evac to float32r ->
      f32r matmul (full PE rate; yraw in PSUM) ->
      ACT Sigmoid(scale=beta_p, bias=beta_p*bias'_p) ->
      GPSIMD m = s*(1-g)+g ->
      DVE o = (yraw + bias'_p) * m  (scalar_tensor_tensor, fused bias add) ->
      PE-transpose back -> DVE residual add (fused PSUM evac) ->
      streamed [128,512] output DMAs.
  - Measured ~189 us/core: ~100% of the 358 GB/s per-core HBM roofline
    (64.5 MiB of unavoidable fp32 I/O per core).
"""
import sys

for _p in ("/opt/trn_rl_repo", "/root/.axon_site/_ro/trn_rl_repo"):
    if _p not in sys.path:
        sys.path.insert(0, _p)

import numpy as np
from contextlib import ExitStack

import concourse.bacc as bacc
import concourse.tile as tile
from concourse import mybir
from concourse import bass_utils

# problem shapes (hardcoded)
BATCH = 16384
IN_SIZE = 4096
N_PART = 32
INT_DIM = 128
N_CORES = 8
ROWS = BATCH // N_CORES          # 2048 rows per core
CHUNK = 512                      # rows per pipeline chunk
N_CHUNK = ROWS // CHUNK          # 4
TPC = CHUNK // 128               # 4 b-tiles (128 rows) per chunk

F32 = mybir.dt.float32
F32R = mybir.dt.float32r
AF = mybir.ActivationFunctionType
ALU = mybir.AluOpType

# engine that computes m = s*(1-gamma)+gamma: "gpsimd" or "vector"
M_ENGINE = "gpsimd"
PREFETCH_P = 28  # p index at which next chunk's loads are emitted


def build_program(repeat=1):
    nc = bacc.Bacc("TRN2", target_bir_lowering=False, debug=False)

    x_d = nc.dram_tensor("x", (ROWS, IN_SIZE), F32, kind="ExternalInput").ap()
    wt_d = nc.dram_tensor("wt", (128, IN_SIZE), F32, kind="ExternalInput").ap()
    cons_d = nc.dram_tensor("cons", (128, 5 * N_PART), F32, kind="ExternalInput").ap()
    id_d = nc.dram_tensor("ident", (128, 128), F32, kind="ExternalInput").ap()
    out_d = nc.dram_tensor("out", (ROWS, IN_SIZE), F32, kind="ExternalOutput").ap()

    with ExitStack() as ctx:
        tc = ctx.enter_context(tile.TileContext(nc))

        # ---- constants: load fp32 scratch, round to f32r, release scratch
        cpool = ctx.enter_context(tc.tile_pool(name="consts", bufs=1))
        with tc.tile_pool(name="scratch", bufs=1) as scratch:
            w_f = scratch.tile([128, IN_SIZE], F32)
            nc.sync.dma_start(w_f[:], wt_d[:])
            wr = cpool.tile([128, IN_SIZE], F32R)
            nc.vector.tensor_copy(wr[:], w_f[:])

        cons_sb = cpool.tile([128, 5 * N_PART], F32)
        nc.sync.dma_start(cons_sb[:], cons_d[:])
        id_sb = cpool.tile([128, 128], F32)
        nc.sync.dma_start(id_sb[:], id_d[:])

        # cons columns: [beta, gamma, 1-gamma, beta*bias1, bias1] per p
        beta_c = lambda p: cons_sb[:, p:p + 1]
        gamma_c = lambda p: cons_sb[:, N_PART + p:N_PART + p + 1]
        omg_c = lambda p: cons_sb[:, 2 * N_PART + p:2 * N_PART + p + 1]
        bb_c = lambda p: cons_sb[:, 3 * N_PART + p:3 * N_PART + p + 1]
        bias_c = lambda p: cons_sb[:, 4 * N_PART + p:4 * N_PART + p + 1]

        # ---- pools
        xpool = ctx.enter_context(tc.tile_pool(name="xin", bufs=8))
        opool = ctx.enter_context(tc.tile_pool(name="oout", bufs=8))
        xtp = ctx.enter_context(tc.tile_pool(name="xt", bufs=4))
        smp = ctx.enter_context(tc.tile_pool(name="sm", bufs=4))
        ogp = ctx.enter_context(tc.tile_pool(name="og", bufs=6))
        psp = ctx.enter_context(tc.tile_pool(name="ps", bufs=3, space="PSUM"))

        m_eng = getattr(nc, M_ENGINE)

        rep_ctx = tc.For_i(0, repeat, 1) if repeat > 1 else None
        if rep_ctx is not None:
            rep_ctx.__enter__()

        def load_chunk(c):
            tiles = [
                xpool.tile([128, IN_SIZE], F32, tag="x", name=f"x_{c}_{i}")
                for i in range(TPC)
            ]
            for qt in range(4):
                c0 = qt * 1024
                for i in range(TPC):
                    r0 = (c * TPC + i) * 128
                    nc.sync.dma_start(tiles[i][:, c0:c0 + 1024],
                                      x_d[r0:r0 + 128, c0:c0 + 1024])
            return tiles

        x_tiles_next = load_chunk(0)
        for c in range(N_CHUNK):
            x_tiles = x_tiles_next
            o_group = []
            for p in range(N_PART):
                if p == PREFETCH_P and c + 1 < N_CHUNK:
                    x_tiles_next = load_chunk(c + 1)
                ps_xt = psp.tile([128, CHUNK], F32, tag="ps_xt", name=f"psxt_{c}_{p}")
                for i in range(TPC):
                    nc.tensor.transpose(
                        ps_xt[:, i * 128:(i + 1) * 128],
                        x_tiles[i][:, p * 128:(p + 1) * 128],
                        id_sb[:],
                    )
                xt_sb = xtp.tile([128, CHUNK], F32R, tag="xts", name=f"xts_{c}_{p}")
                nc.scalar.copy(xt_sb[:], ps_xt[:])

                ps_y = psp.tile([128, CHUNK], F32, tag="ps_y", name=f"psy_{c}_{p}")
                nc.tensor.matmul(ps_y[:], wr[:, p * 128:(p + 1) * 128], xt_sb[:],
                                 start=True, stop=True)

                # s = sigmoid(beta*(yraw + bias1)) = sigmoid(beta*yraw + beta*bias1)
                s_sb = smp.tile([128, CHUNK], F32, tag="s", name=f"s_{c}_{p}")
                nc.scalar.activation(s_sb[:], ps_y[:], AF.Sigmoid,
                                     scale=beta_c(p), bias=bb_c(p))

                m_sb = smp.tile([128, CHUNK], F32, tag="m", name=f"m_{c}_{p}")
                m_eng.tensor_scalar(m_sb[:], s_sb[:], omg_c(p), gamma_c(p),
                                    ALU.mult, ALU.add)

                # o = (yraw + bias1) * m   (fused bias add + gate)
                o_sb = ogp.tile([128, CHUNK], F32, tag="o", name=f"o_{c}_{p}")
                nc.vector.scalar_tensor_tensor(o_sb[:], ps_y[:], bias_c(p), m_sb[:],
                                               ALU.add, ALU.mult)

                o_group.append(o_sb)
                if p % 4 == 3:
                    q = p // 4
                    for i in range(TPC):
                        ps_og = psp.tile([128, 512], F32, tag="ps_og", bufs=2,
                                         name=f"psog_{c}_{q}_{i}")
                        for j in range(4):
                            nc.tensor.transpose(
                                ps_og[:, j * 128:(j + 1) * 128],
                                o_group[j][:, i * 128:(i + 1) * 128],
                                id_sb[:],
                            )
                        res_sb = opool.tile([128, 512], F32, tag="res",
                                            name=f"res_{c}_{q}_{i}")
                        nc.vector.tensor_tensor(
                            res_sb[:],
                            ps_og[:],
                            x_tiles[i][:, q * 512:(q + 1) * 512],
                            ALU.add,
                        )
                        r0 = (c * TPC + i) * 128
                        nc.sync.dma_start(
                            out_d[r0:r0 + 128, q * 512:(q + 1) * 512], res_sb[:])
                    o_group = []

        if rep_ctx is not None:
            rep_ctx.__exit__(None, None, None)

    nc.finalize()
    return nc


def fold_constants(weights, bias, gain, norm_bias, gamma, beta):
    W = weights * float(np.reshape(gain, -1)[0])                  # [P, D, D]
    colsum = W.sum(axis=1)                                        # [P, D]
    bias1 = (bias + float(np.reshape(norm_bias, -1)[0]) * colsum.reshape(-1)).astype(np.float32)
    wt = np.ascontiguousarray(W.transpose(1, 0, 2).reshape(128, IN_SIZE)).astype(np.float32)
    beta_p = beta.reshape(N_PART, 128).T                          # [128, P]
    gamma_p = gamma.reshape(N_PART, 128).T
    omg_p = (1.0 - gamma).reshape(N_PART, 128).T
    bb_p = (beta * bias1).reshape(N_PART, 128).T
    bias_p = bias1.reshape(N_PART, 128).T
    cons = np.ascontiguousarray(
        np.concatenate([beta_p, gamma_p, omg_p, bb_p, bias_p], axis=1).astype(np.float32))
    ident = np.eye(128, dtype=np.float32)
    return wt, cons, ident


def kernel(x, weights, bias, gain, norm_bias, gamma, beta, **_ignored):
    x = np.ascontiguousarray(np.asarray(x, dtype=np.float32))
    weights = np.asarray(weights, dtype=np.float32)
    bias = np.asarray(bias, dtype=np.float32)
    gain = np.asarray(gain, dtype=np.float32)
    norm_bias = np.asarray(norm_bias, dtype=np.float32)
    gamma = np.asarray(gamma, dtype=np.float32)
    beta = np.asarray(beta, dtype=np.float32)

    # host-side constant folding
    wt, cons, ident = fold_constants(weights, bias, gain, norm_bias, gamma, beta)

    nc = build_program()

    in_maps = []
    for core in range(N_CORES):
        shard = x[core * ROWS:(core + 1) * ROWS]
        in_maps.append({
            "x": shard,
            "wt": wt,
            "cons": cons,
            "ident": ident,
        })

    res = bass_utils.run_bass_kernel_spmd(nc, in_maps, core_ids=list(range(N_CORES)))
    out = np.concatenate([res.results[i]["out"] for i in range(N_CORES)], axis=0)
    return out


if __name__ == "__main__":
    xs = np.random.randn(BATCH, IN_SIZE).astype(np.float32)
    ws = np.random.randn(N_PART, INT_DIM, INT_DIM).astype(np.float32) / 11.3
    out = kernel(
        x=xs, weights=ws,
        bias=np.zeros(IN_SIZE, np.float32),
        gain=np.ones(1, np.float32),
        norm_bias=np.zeros(1, np.float32),
        gamma=np.ones(IN_SIZE, np.float32),
        beta=np.zeros(IN_SIZE, np.float32),
    )
    print(out.shape, out.dtype)
